# revision 10
# baseline (speedup 1.0000x reference)
"""TRN2 Bass kernel for nn_BatchedCauchyKernel3d.

reference:
    d   = clip(||x_n||^2 + ||y_m||^2 - 2 x_n.y_m, 1e-10, 1e6)
    sxy = sqrt(clip(scale_x_n * scale_y_m, 1e-10, 1e12))
    out = 1 / (1 + d / sxy)

Rewrite: with u_n = sqrt(scale_x_n), v_m = sqrt(scale_y_m):
    1 + d/sxy = sum_k XA[k,n] * YA[k,m]      (K = 6 augmented contraction)
      XA = [-2 x1/u, -2 x2/u, -2 x3/u, ||x||^2/u, 1/u, 1]
      YA = [   y1/v,    y2/v,    y3/v,       1/v, ||y||^2/v, 1]
so the whole kernel matrix is ONE matmul followed by an elementwise
reciprocal.  The matmul runs in bf16 with a 3-way hi/mid/lo split of each
operand (6 cross-term pairs -> K = 36), which reproduces fp32 accuracy at
full (1 col/cycle) PE speed; fp32-native matmuls are 4x slower on TRN2.

Sharding: 8 cores, core c owns batch c//2, row half c%2 -> a (2048, 4096)
f32 output block per core (the output DMA of 32 MB/core is the roofline).
"""

import sys

if "/opt/trn_rl_repo" not in sys.path:
    sys.path.insert(0, "/opt/trn_rl_repo")

import numpy as np

B, NX, NY, FDIM = 4, 4096, 4096, 16
NCORES = 8
R = B * NX // NCORES  # 2048 rows per core
KPAIRS = 6  # (h,h),(h,m),(m,h),(h,l),(m,m),(l,h)
KR = 6 * KPAIRS  # 36

_CACHE = {}


def _build_program(rows, ny):
    from contextlib import ExitStack

    import concourse.tile as tile
    from concourse import bacc, mybir

    BF16 = mybir.dt.bfloat16
    F32 = mybir.dt.float32

    NB = 512  # matmul moving free dim (one PSUM bank of fp32)
    CH = 2048  # reciprocal chunk = 4 PSUM banks

    # input carries the 36 contraction rows twice (partitions 0-35 and
    # 64-99) so matmuls can alternate PE row-groups and run concurrently
    nc = bacc.Bacc("TRN2", target_bir_lowering=False, debug=False)
    xya = nc.declare_dram_parameter("xya", [64 + KR, rows + ny], BF16, isOutput=False)
    out = nc.declare_dram_parameter("out", [rows, ny], F32, isOutput=True)

    with ExitStack() as ctx:
        tc = ctx.enter_context(tile.TileContext(nc))
        const = ctx.enter_context(tc.tile_pool(name="const", bufs=1))
        psum = ctx.enter_context(tc.tile_pool(name="psum", bufs=2, space="PSUM"))
        outp = ctx.enter_context(tc.tile_pool(name="outp", bufs=6))

        # input loads issued from the (otherwise idle) Scalar engine's HWDGE
        # path: the Sync engine spends the first ~7us of the kernel on the
        # framework preamble and would delay the first transfer.  Split so
        # the first matmuls only wait on the slices they read.
        xya_sb = const.tile([64 + KR, rows + ny], BF16)
        nc.scalar.dma_start(xya_sb[:, 0:rows], xya[:, 0:rows])
        nc.scalar.dma_start(xya_sb[:, rows : rows + CH], xya[:, rows : rows + CH])
        nc.scalar.dma_start(xya_sb[:, rows + CH :], xya[:, rows + CH :])

        for m in range(rows // 128):
            for h in range(ny // CH):
                ps = psum.tile([128, CH], F32, tag="ps")
                for j in range(CH // NB):
                    col = h * CH + j * NB
                    g = 64 * (j % 2)
                    nc.tensor.matmul(
                        ps[:, j * NB : (j + 1) * NB],
                        xya_sb[g : g + KR, m * 128 : (m + 1) * 128],
                        xya_sb[g : g + KR, rows + col : rows + col + NB],
                        start=True,
                        stop=True,
                        tile_position=(g, 0),
                    )
                # fine-grained epilogue for the first row-tile so output DMA
                # starts as early as possible
                ot = outp.tile([128, CH], F32)
                if m == 0:
                    for j in range(CH // NB):
                        sl = slice(j * NB, (j + 1) * NB)
                        nc.vector.reciprocal_approx_fast(out=ot[:, sl], in_=ps[:, sl])
                        nc.sync.dma_start(
                            out[0:128, h * CH + j * NB : h * CH + (j + 1) * NB],
                            ot[:, sl],
                        )
                else:
                    nc.vector.reciprocal_approx_fast(out=ot[:], in_=ps[:])
                    nc.sync.dma_start(
                        out[m * 128 : (m + 1) * 128, h * CH : (h + 1) * CH], ot[:]
                    )

    nc.compile()
    return nc


def _get_program(rows=R, ny=NY):
    key = (rows, ny)
    if key not in _CACHE:
        _CACHE[key] = _build_program(rows, ny)
    return _CACHE[key]


def _augment(x, y, sample_x, sample_y, scale):
    """Host-side O(N) prep: augmented (B,6,NX) / (B,6,NY) factor matrices."""
    s = np.clip(scale.astype(np.float64), 1e-6, 1e6)
    sx = np.clip(sample_x.astype(np.float64) @ s, 1e-10, 1e6)  # (B,NX)
    sy = np.clip(sample_y.astype(np.float64) @ s, 1e-10, 1e6)  # (B,NY)
    u = np.sqrt(sx)
    v = np.sqrt(sy)
    x64 = x.astype(np.float64)
    y64 = y.astype(np.float64)
    sqx = (x64 * x64).sum(-1)
    sqy = (y64 * y64).sum(-1)
    one_x = np.ones_like(u)
    XA = np.stack(
        [
            -2.0 * x64[..., 0] / u,
            -2.0 * x64[..., 1] / u,
            -2.0 * x64[..., 2] / u,
            sqx / u,
            1.0 / u,
            one_x,
        ],
        axis=1,
    )  # (B, 6, NX)
    YA = np.stack(
        [
            y64[..., 0] / v,
            y64[..., 1] / v,
            y64[..., 2] / v,
            1.0 / v,
            sqy / v,
            np.ones_like(v),
        ],
        axis=1,
    )  # (B, 6, NY)
    return XA, YA


def _split3(a64):
    """float64 (B,6,L) -> three bf16 (B,6,L) planes: hi, mid, lo."""
    import ml_dtypes

    bf = ml_dtypes.bfloat16
    a32 = a64.astype(np.float32)
    h = a32.astype(bf)
    r1 = a32 - h.astype(np.float32)
    m = r1.astype(bf)
    r2 = r1 - m.astype(np.float32)
    l = r2.astype(bf)
    return h, m, l


def _pack_rows(x, y, sample_x, sample_y, scale):
    """Returns per-core packed (KR, R+NY) bf16 inputs."""
    XA, YA = _augment(x, y, sample_x, sample_y, scale)
    xh, xm, xl = _split3(XA)
    yh, ym, yl = _split3(YA)
    # 6 cross-term pairs capturing (hi+mid+lo)x(hi+mid+lo) down to 2^-24
    XROWS = np.concatenate([xh, xh, xm, xh, xm, xl], axis=1)  # (B, 36, NX)
    YROWS = np.concatenate([yh, ym, yh, yl, ym, yh], axis=1)  # (B, 36, NY)
    ins = []
    for c in range(NCORES):
        b, half = divmod(c, NCORES // B)
        xa_c = XROWS[b][:, half * R : (half + 1) * R]
        rows36 = np.concatenate([xa_c, YROWS[b]], axis=1)  # (36, R+NY)
        pad = np.zeros((64 - KR, rows36.shape[1]), dtype=rows36.dtype)
        ins.append(np.ascontiguousarray(np.concatenate([rows36, pad, rows36], axis=0)))
    return ins


def _run(inputs, trace=False):
    from concourse.bass_utils import run_bass_kernel_spmd

    ins = _pack_rows(
        inputs["x"], inputs["y"], inputs["sample_x"], inputs["sample_y"], inputs["scale"]
    )
    nc = _get_program()
    in_maps = [{"xya": a} for a in ins]
    res = run_bass_kernel_spmd(nc, in_maps, list(range(NCORES)), trace=trace)
    out = np.empty((B, NX, NY), dtype=np.float32)
    for c in range(NCORES):
        b, half = divmod(c, NCORES // B)
        out[b, half * R : (half + 1) * R, :] = res.results[c]["out"]
    return out, res


def kernel(x, y, sample_x, sample_y, scale):
    out, _ = _run(
        {"x": x, "y": y, "sample_x": sample_x, "sample_y": sample_y, "scale": scale}
    )
    return out
